# revision 7
# baseline (speedup 1.0000x reference)
"""Trainium2 Bass kernel for CompressedGlobalAttention (v3, bf16, pipelined).

Same math/sharding/masking as v2 (see kernel2.py docstring).  v3
restructures phase B into a software pipeline so the PE is never idle
(the PE's clock drops to a low pstate whenever its pipeline gaps, which
made v2 run matmuls at ~1.2 GHz instead of 2.4 GHz):

  - the attention inner loop over heads (Act-exp-bound) is interleaved
    with the q-projection matmuls of seq-tile st+1 and the
    normalization + output-projection matmuls of seq-tile st-1, all of
    which are Act-light, so PE and Act both stay busy;
  - attn@v for head h is issued one head behind the score matmuls so
    exp(h) overlaps the next head's scores;
  - elementwise work is split across engines: exp + bias adds on Act,
    PSUM drains + normalization muls + reciprocals on DVE, pooling +
    boundary-mask adds + vaug/ysb copies on GpSimd;
  - the softmax-denominator clamp is dropped (a zero denominator can
    only occur for query rows 0..7 of the batch, which the host
    overwrites with the analytic uniform-attention value anyway);
  - the pooling input x^T streams through SBUF chunk-by-chunk instead
    of being fully resident.
"""

import os
import sys

import numpy as np

NUM_HEADS = 16
HEAD_DIM = 64
RATIO = 8
B, S, D = 2, 8192, 1024
LWS = 4096
NPOOL = LWS // RATIO        # 512
SQ = S // 4                 # 2048 query rows per core
N_CORES = 8
ST = 512                    # seq tile (free dim) in phase B
NST = SQ // ST              # 4 seq tiles per core
NEG = -1.0e9

_RUNNER = None


def _ensure_path():
    for p in ("/opt/trn_rl_repo",):
        if p not in sys.path and os.path.isdir(p):
            sys.path.insert(0, p)


def build_program():
    """Build the Bass/Tile SPMD program (same for all 8 cores)."""
    _ensure_path()
    import concourse.bacc as bacc
    import concourse.mybir as mybir
    import concourse.tile as tile
    from contextlib import ExitStack

    f32 = mybir.dt.float32
    bf16 = mybir.dt.bfloat16
    Exp = mybir.ActivationFunctionType.Exp

    nc = bacc.Bacc("TRN2", target_bir_lowering=False, debug=False)

    # xqt: [st, p, m*ST+s'] so each seq-tile loads with ONE contiguous DMA
    xqt = nc.declare_dram_parameter("xqt", [NST * 128, 8 * ST], bf16, isOutput=False)
    # xpn: [p, chunk*D+d] natural window rows packed so s = 128*chunk + p
    xpn = nc.declare_dram_parameter("xpn", [128, (LWS // 128) * D], bf16, isOutput=False)
    p8d = nc.declare_dram_parameter("p8", [128, 8 * 128], bf16, isOutput=False)
    # weights: [p, m*D+c] (8 row-chunks side by side) -> ONE 16KB/row DMA each
    wq = nc.declare_dram_parameter("wq", [128, 8 * D], bf16, isOutput=False)
    wk = nc.declare_dram_parameter("wk", [128, 8 * D], bf16, isOutput=False)
    wv = nc.declare_dram_parameter("wv", [128, 8 * D], bf16, isOutput=False)
    wo = nc.declare_dram_parameter("wo", [128, 8 * D], bf16, isOutput=False)
    bq2 = nc.declare_dram_parameter("bq2", [128, 8], f32, isOutput=False)
    bk2 = nc.declare_dram_parameter("bk2", [128, 8], f32, isOutput=False)
    hsd = nc.declare_dram_parameter("headsel", [16, D], bf16, isOutput=False)
    dgd = nc.declare_dram_parameter("diagmask", [64, NST * ST], f32, isOutput=False)
    bmd = nc.declare_dram_parameter("biasmask", [128, 16], f32, isOutput=False)
    yout = nc.declare_dram_parameter("y", [SQ, D], f32, isOutput=True)

    with tile.TileContext(nc) as tc, ExitStack() as top, \
            nc.allow_low_precision(reason="bf16 kernel by design; fp32 PSUM accum"):
        # ---------------- persistent pools ----------------
        consts = top.enter_context(tc.tile_pool(name="consts", bufs=1))
        kTp = top.enter_context(tc.tile_pool(name="kTp", bufs=1))
        vap = top.enter_context(tc.tile_pool(name="vap", bufs=1))
        wqop = top.enter_context(tc.tile_pool(name="wqop", bufs=1))
        xTp = top.enter_context(tc.tile_pool(name="xTp", bufs=2))
        qTp = top.enter_context(tc.tile_pool(name="qTp", bufs=2))
        oTp = top.enter_context(tc.tile_pool(name="oTp", bufs=2))
        ep = top.enter_context(tc.tile_pool(name="ep", bufs=2))
        dnp = top.enter_context(tc.tile_pool(name="dnp", bufs=2))
        ysp = top.enter_context(tc.tile_pool(name="ysp", bufs=2))
        psb = top.enter_context(tc.tile_pool(name="psb", bufs=1, space="PSUM"))

        headsel = consts.tile([16, D], bf16, name="headsel")
        bq2_sb = consts.tile([128, 8], f32, name="bq2_sb")
        diag_sb = consts.tile([64, NST * ST], f32, name="diag_sb")
        bias_sb = consts.tile([128, 16], f32, name="bias_sb")

        def dma_consts():
            nc.sync.dma_start(headsel[:], hsd[:, :])
            nc.sync.dma_start(bq2_sb[:], bq2[:, :])
            nc.sync.dma_start(diag_sb[:], dgd[:, :])
            nc.sync.dma_start(bias_sb[:], bmd[:, :])

        wqall = wqop.tile([128, 8 * D], bf16, name="wqall")
        woall = wqop.tile([128, 8 * D], bf16, name="woall")
        wq_sb = [wqall[:, m * D : (m + 1) * D] for m in range(8)]
        wo_sb = [woall[:, j * D : (j + 1) * D] for j in range(8)]

        kT = [kTp.tile([128, NPOOL], bf16, name=f"kT{j}", tag=f"kT{j}") for j in range(8)]
        vaug = [
            vap.tile([128, NUM_HEADS * (HEAD_DIM + 1)], bf16, name=f"vaug{i}", tag=f"vaug{i}")
            for i in range(4)
        ]

        # per-st tile state
        xT = {}     # st -> [8] tiles [128, ST] bf16
        qT = {}     # st -> [8] tiles
        oT = {}     # st -> [8] tiles
        den = {}    # st -> denoms tile [16, ST] f32
        etile = {}  # (st, h) -> [4] e tiles

        def dma_xT(st):
            xTall = xTp.tile([128, 8 * ST], bf16, name="xTall", tag="xTall")
            nc.sync.dma_start(xTall[:], xqt[st * 128 : (st + 1) * 128, :])
            xT[st] = [xTall[:, m * ST : (m + 1) * ST] for m in range(8)]

        # ---- qT units: one closure per (j, m) matmul; m==7 adds bias+cast ----
        def make_qt_units(st):
            qT[st] = [qTp.tile([128, ST], bf16, name=f"qT{j}", tag=f"qT{j}") for j in range(8)]
            state = {}

            def unit(j, m):
                def run():
                    if m == 0:
                        state[j] = psb.tile([128, ST], f32, name="qps", tag="qr", bufs=2)
                    nc.tensor.matmul(
                        state[j][:],
                        wq_sb[m][:, j * 128 : (j + 1) * 128],
                        xT[st][m][:],
                        start=(m == 0),
                        stop=(m == 7),
                    )
                    if m == 7:
                        nc.scalar.add(qT[st][j][:], state[j][:], bq2_sb[:, j : j + 1])
                return run

            return [unit(j, m) for j in range(8) for m in range(8)]

        # ---- normalization + yproj units for st (run during st+1's loop) ----
        def make_yp_units(st):
            s0 = st * ST
            units = []

            def recips_unit():
                r = dnp.tile([16, ST], bf16, name="recips", tag="recips", bufs=2)
                den[(st, "recips")] = r
                nc.vector.reciprocal(r[:], den[st][:])
            units.append(recips_unit)

            def norm_unit(j):
                def run():
                    r = den[(st, "recips")]
                    rps = psb.tile([128, ST], f32, name="rps", tag="rps", bufs=1)
                    nc.tensor.matmul(
                        rps[:],
                        headsel[:, j * 128 : (j + 1) * 128],
                        r[:],
                        start=True,
                        stop=True,
                    )
                    nc.vector.tensor_mul(oT[st][j][:], oT[st][j][:], rps[:])
                return run
            units.extend(norm_unit(j) for j in range(8))

            # 8 half-groups of yproj: (q4, hf) -> 9 matmuls + drain/DMA
            ystate = {}

            def yp_half(q4, hf, lo, hi, finish):
                def run():
                    if lo == 0:
                        ystate[(q4, hf)] = psb.tile(
                            [128, 512], f32, name=f"y{hf}", tag=f"yh{hf}", bufs=1
                        )
                    t = ystate[(q4, hf)]
                    for j in range(lo, hi):
                        nc.tensor.matmul(
                            t[:],
                            oT[st][j][:, q4 * 128 : (q4 + 1) * 128],
                            wo_sb[j][:, hf * 512 : (hf + 1) * 512],
                            start=(j == 0),
                            stop=(j == 7),
                        )
                    if finish:
                        ysb = ystate.setdefault(
                            q4, ysp.tile([128, D], f32, name="ysb", tag="ysb", bufs=2)
                        )
                        nc.vector.tensor_copy(ysb[:, hf * 512 : (hf + 1) * 512], t[:])
                        if hf == 1:
                            nc.sync.dma_start(
                                yout[s0 + q4 * 128 : s0 + q4 * 128 + 128, :], ysb[:]
                            )
                return run

            for q4 in range(4):
                units.append(yp_half(q4, 0, 0, 4, False))
                units.append(yp_half(q4, 0, 4, 8, True))
                units.append(yp_half(q4, 1, 0, 4, False))
                units.append(yp_half(q4, 1, 4, 8, True))
            return units

        # ---- attention pieces ----
        def emit_scores_half(st, h, pcs):
            """Two score matmuls + their exps; fillers go between halves so
            the PE never waits on the exp draining the ping-pong PSUM bank."""
            j, r0 = h // 2, 64 * (h % 2)
            dpc = st // 2
            dof = 64 * (st % 2)
            es = etile.setdefault((st, h), [None] * 4)
            for pc in pcs:
                t = psb.tile([128, ST], f32, name=f"sc{pc}", tag=f"pc{pc % 2}", bufs=1)
                nc.tensor.matmul(
                    t[:],
                    kT[j][r0 : r0 + 64, pc * 128 : (pc + 1) * 128],
                    qT[st][j][r0 : r0 + 64, :],
                    start=True,
                    stop=True,
                )
                if pc == dpc:
                    nc.vector.tensor_add(
                        t[dof : dof + 64, :],
                        t[dof : dof + 64, :],
                        diag_sb[:, st * ST : (st + 1) * ST],
                    )
                et = ep.tile([128, ST], bf16, name=f"e{pc}", tag=f"e{pc}")
                nc.scalar.activation(
                    et[:],
                    t[:],
                    Exp,
                    bias=bias_sb[:, st * 4 + pc : st * 4 + pc + 1],
                    scale=1.0 / np.sqrt(HEAD_DIM),
                )
                es[pc] = et

        def emit_attnv(st, h):
            j, r0 = h // 2, 64 * (h % 2)
            es = etile.pop((st, h))
            oa = psb.tile([HEAD_DIM + 1, ST], f32, name="oa", tag="oa", bufs=1)
            for pc in range(4):
                nc.tensor.matmul(
                    oa[:],
                    vaug[pc][:, h * 65 : h * 65 + 65],
                    es[pc][:],
                    start=(pc == 0),
                    stop=(pc == 3),
                )
            nc.vector.tensor_copy(oT[st][j][r0 : r0 + 64, :], oa[0:HEAD_DIM, :])
            drow = dnp.tile([1, ST], bf16, name="drow", tag="drow", bufs=4)
            nc.vector.tensor_copy(drow[:], oa[HEAD_DIM : HEAD_DIM + 1, :])
            nc.sync.dma_start(den[st][h : h + 1, :], drow[:])

        # ================= program =================
        # DMA priority: xT(0) + wq feed qT(0); xpt feeds pooling/kT;
        # consts + wo are needed later
        dma_xT(0)
        nc.sync.dma_start(wqall[:], wq[:, :])

        # ---- phase A: pooled k/v (+ qT(0) first so the PE starts early) ----
        with ExitStack() as pa:
            aconsts = pa.enter_context(tc.tile_pool(name="aconsts", bufs=1))
            wkvp = pa.enter_context(tc.tile_pool(name="wkvp", bufs=1))
            xptp = pa.enter_context(tc.tile_pool(name="xptp", bufs=2))
            pltp = pa.enter_context(tc.tile_pool(name="pltp", bufs=1))

            wkall = wkvp.tile([128, 8 * D], bf16, name="wkall")
            wvall = wkvp.tile([128, 8 * D], bf16, name="wvall")
            wk_sb = [wkall[:, m * D : (m + 1) * D] for m in range(8)]
            wv_sb = [wvall[:, m * D : (m + 1) * D] for m in range(8)]

            # sum-pooling on the PE (1/R folded into Wk/Wv): per pool group
            # g, accumulate 8 s-chunks through the sparse p8 selector, then
            # PE-transpose pooled [pools, d] into pooledT [d, pools].
            # xpn streams as 8 pieces of [128, 4*D] (8KB rows, 4 s-chunks).
            from concourse.masks import make_identity

            p8_sb = aconsts.tile([128, 8 * 128], bf16, name="p8_sb")
            nc.sync.dma_start(p8_sb[:], p8d[:, :])
            ident = aconsts.tile([128, 128], bf16, name="ident")
            make_identity(nc, ident[:])

            pooledT = [pltp.tile([128, NPOOL], bf16, name=f"pooledT{m}", tag=f"pooledT{m}") for m in range(8)]
            pieces = {}

            def get_piece(piece):
                if piece not in pieces:
                    xc = xptp.tile([128, 4 * D], bf16, name="xpnp", tag="xpnp", bufs=2)
                    nc.sync.dma_start(
                        xc[:], xpn[:, piece * 4 * D : (piece + 1) * 4 * D]
                    )
                    pieces[piece] = xc
                return pieces[piece]

            for g in range(4):
                ps2 = [
                    psb.tile([128, 512], f32, name=f"plps{h2}", tag=f"yh{h2}", bufs=1)
                    for h2 in range(2)
                ]
                for c in range(8):
                    chunk = 8 * g + c
                    xc = get_piece(chunk // 4)
                    col0 = (chunk % 4) * D
                    for h2 in range(2):
                        nc.tensor.matmul(
                            ps2[h2][:],
                            p8_sb[:, c * 128 : (c + 1) * 128],
                            xc[:, col0 + h2 * 512 : col0 + (h2 + 1) * 512],
                            start=(c == 0),
                            stop=(c == 7),
                        )
                pooled_sb = xptp.tile([128, D], bf16, name="pooled_sb", tag="pooled_sb", bufs=2)
                for h2 in range(2):
                    nc.vector.tensor_copy(
                        pooled_sb[:, h2 * 512 : (h2 + 1) * 512], ps2[h2][:]
                    )
                for m in range(8):
                    tp = psb.tile([128, 512], bf16, name="trp", tag=f"pc{m % 2}", bufs=1)
                    nc.tensor.transpose(
                        tp[:, 0:128], pooled_sb[:, m * 128 : (m + 1) * 128], ident[:]
                    )
                    nc.vector.tensor_copy(
                        pooledT[m][:, g * 128 : (g + 1) * 128], tp[:, 0:128]
                    )

            nc.sync.dma_start(wkall[:], wk[:, :])
            nc.sync.dma_start(wvall[:], wv[:, :])
            dma_consts()
            nc.sync.dma_start(woall[:], wo[:, :])
            bk2_sb = aconsts.tile([128, 8], f32, name="bk2_sb")
            nc.sync.dma_start(bk2_sb[:], bk2[:, :])

            # qT(0) ahead of k/v so the PE has work as soon as wq+xT(0) land
            for u in make_qt_units(0):
                u()

            for j in range(8):
                ps = psb.tile([128, NPOOL], f32, name="ps2", tag="qr", bufs=2)
                for m in range(8):
                    nc.tensor.matmul(
                        ps[:],
                        wk_sb[m][:, j * 128 : (j + 1) * 128],
                        pooledT[m][:],
                        start=(m == 0),
                        stop=(m == 7),
                    )
                nc.scalar.add(kT[j][:], ps[:], bk2_sb[:, j : j + 1])

            for i in range(4):
                psh = [
                    psb.tile([128, 512], f32, name=f"ps3{h2}", tag=f"yh{h2}", bufs=1)
                    for h2 in range(2)
                ]
                for m in range(8):
                    for h2 in range(2):
                        nc.tensor.matmul(
                            psh[h2][:],
                            pooledT[m][:, i * 128 : (i + 1) * 128],
                            wv_sb[m][:, h2 * 512 : (h2 + 1) * 512],
                            start=(m == 0),
                            stop=(m == 7),
                        )
                va = vaug[i][:].rearrange("p (h x) -> p h x", x=HEAD_DIM + 1)
                for h2 in range(2):
                    nc.vector.tensor_copy(
                        va[:, 8 * h2 : 8 * (h2 + 1), 0:HEAD_DIM],
                        psh[h2][:].rearrange("p (h x) -> p h x", x=HEAD_DIM),
                    )
                nc.gpsimd.memset(va[:, :, HEAD_DIM : HEAD_DIM + 1], 1.0)

        # ---- phase B: pipelined st loop ----
        for st in range(NST):
            if st + 1 < NST:
                dma_xT(st + 1)
            oT[st] = [oTp.tile([128, ST], bf16, name=f"oT{j}", tag=f"oT{j}") for j in range(8)]
            den[st] = dnp.tile([16, ST], bf16, name="denoms", tag="denoms", bufs=2)

            qt_units = make_qt_units(st + 1) if st + 1 < NST else []
            yp_units = make_yp_units(st - 1) if st >= 1 else []

            qi_ = 0
            yi = 0
            for h in range(NUM_HEADS):
                emit_scores_half(st, h, [0, 1])
                for _ in range(2):
                    if qi_ < len(qt_units):
                        qt_units[qi_]()
                        qi_ += 1
                emit_scores_half(st, h, [2, 3])
                if h > 0:
                    emit_attnv(st, h - 1)
                elif st > 0:
                    emit_attnv(st - 1, NUM_HEADS - 1)
                for _ in range(2):
                    if qi_ < len(qt_units):
                        qt_units[qi_]()
                        qi_ += 1
                # filler: norm/yproj(st-1) units
                ntake = ((h + 1) * len(yp_units)) // NUM_HEADS - yi
                for _ in range(ntake):
                    yp_units[yi]()
                    yi += 1
            while qi_ < len(qt_units):
                qt_units[qi_]()
                qi_ += 1
            while yi < len(yp_units):
                yp_units[yi]()
                yi += 1

        # epilogue: last head + normalization + yproj of st=3
        emit_attnv(NST - 1, NUM_HEADS - 1)
        for u in make_yp_units(NST - 1):
            u()

    nc.finalize()
    return nc


# ---------------------------------------------------------------------------
# host side
# ---------------------------------------------------------------------------

def _p8_selector():
    import ml_dtypes
    p8 = np.zeros((128, 8, 128), np.float32)
    for p in range(128):
        for c in range(8):
            p8[p, c, 16 * c + p // 8] = 1.0
    return np.ascontiguousarray(p8.reshape(128, 8 * 128).astype(ml_dtypes.bfloat16))


def _host_constants():
    headsel = np.zeros((16, D), np.float32)
    for h in range(16):
        headsel[h, h * 64 : (h + 1) * 64] = 1.0
    return headsel


def _core_masks(qi):
    """diagmask (64, NST*ST) and biasmask (128, 16) for quarter qi."""
    diag = np.zeros((64, NST, ST), np.float32)
    if qi < 2:
        for st in range(NST):
            stg = 4 * qi + st
            pg = 64 * stg + np.arange(64)[:, None]
            sg = qi * SQ + st * ST + np.arange(ST)[None, :]
            diag[:, st, :] = np.where(sg >= 8 * pg + 8, 0.0, NEG)
    diag = np.ascontiguousarray(diag.reshape(64, NST * ST))

    bias = np.zeros((128, 16), np.float32)
    for st in range(NST):
        for pc in range(4):
            for pl in range(128):
                pp = 128 * pc + pl
                if qi == 1:
                    porig = pp + 256 if pp < 256 else pp - 256
                else:
                    porig = pp
                in_strip = qi < 2 and (64 * st <= pp < 64 * st + 64)
                if in_strip:
                    val = 0.0
                else:
                    s_min = qi * SQ + st * ST
                    val = 0.0 if s_min >= 8 * porig + 8 else NEG
                bias[pl, st * 4 + pc] = val
    return diag, bias


def _numpy_reference(x, lws, Wq, bq, Wk, bk, Wv, bv, Wo, bo):
    Bx, Sx, Dx = x.shape
    H, Hd, R = NUM_HEADS, HEAD_DIM, RATIO
    if lws <= R:
        return np.zeros_like(x)
    npool = lws // R
    trunc = npool * R
    comp = x[:, :trunc, :].reshape(Bx, npool, R, Dx).mean(axis=2)
    q = (x @ Wq + bq).reshape(Bx, Sx, H, Hd).transpose(0, 2, 1, 3)
    k = (comp @ Wk + bk).reshape(Bx, npool, H, Hd).transpose(0, 2, 1, 3)
    v = (comp @ Wv + bv).reshape(Bx, npool, H, Hd).transpose(0, 2, 1, 3)
    scores = np.einsum("bhqd,bhkd->bhqk", q, k) / np.sqrt(Hd)
    mask = np.arange(Sx)[:, None] >= (np.arange(npool) + 1) * R
    scores = np.where(mask[None, None], scores, -1e9)
    scores = scores - scores.max(axis=-1, keepdims=True)
    e = np.exp(scores)
    attn = e / e.sum(axis=-1, keepdims=True)
    out = np.einsum("bhqk,bhkd->bhqd", attn, v)
    out = out.transpose(0, 2, 1, 3).reshape(Bx, Sx, H * Hd)
    return (out @ Wo + bo).astype(np.float32)


def make_in_maps(x, Wq, bq, Wk, bk, Wv, bv, Wo, bo):
    import ml_dtypes

    bf = ml_dtypes.bfloat16
    x = np.asarray(x, np.float32)
    headsel = _host_constants().astype(bf)
    def pack_w(w):
        # [p, m*D+c] row-chunk-major so the whole matrix is one 16KB/row DMA
        return np.ascontiguousarray(
            w.astype(bf).reshape(8, 128, D).transpose(1, 0, 2).reshape(128, 8 * D)
        )

    wqb = pack_w(np.asarray(Wq, np.float32))
    # device pools by SUM; fold the 1/RATIO average into Wk/Wv
    wkb = pack_w(np.asarray(Wk, np.float32) / RATIO)
    wvb = pack_w(np.asarray(Wv, np.float32) / RATIO)
    wob = pack_w(np.asarray(Wo, np.float32))
    bq2 = np.ascontiguousarray(np.asarray(bq, np.float32).reshape(8, 128).T)
    bk2 = np.ascontiguousarray(np.asarray(bk, np.float32).reshape(8, 128).T)

    xb = [x[b].astype(bf) for b in range(B)]  # (S, D) bf16 per batch
    xTb = [np.ascontiguousarray(xb[b].T) for b in range(B)]  # (D, S)

    in_maps = []
    for core in range(N_CORES):
        b, qi = core // 4, core % 4
        # [st*128+p, m*ST+s'] so each seq-tile is one contiguous 8KB/row DMA
        xqt = np.ascontiguousarray(
            xTb[b][:, qi * SQ : (qi + 1) * SQ]
            .reshape(8, 128, NST, ST).transpose(2, 1, 0, 3).reshape(NST * 128, 8 * ST)
        )
        if qi == 1:
            xwin = np.concatenate([xb[b][2048:4096], xb[b][0:2048]], axis=0)
        else:
            xwin = xb[b][:LWS]
        xpnc = np.ascontiguousarray(
            xwin.reshape(LWS // 128, 128, D).transpose(1, 0, 2).reshape(128, (LWS // 128) * D)
        )
        diag, bias = _core_masks(qi)
        in_maps.append(
            {
                "xqt": xqt,
                "xpn": xpnc,
                "p8": _p8_selector(),
                "wq": wqb,
                "wk": wkb,
                "wv": wvb,
                "wo": wob,
                "bq2": bq2,
                "bk2": bk2,
                "headsel": headsel,
                "diagmask": diag,
                "biasmask": bias,
            }
        )
    return in_maps


def assemble_output(x, Wv, bv, Wo, bo, results):
    y = np.empty((B, S, D), np.float32)
    # device output omits the constant (bv @ Wo + bo) row (softmax weights
    # sum to 1, so bv contributes a constant through Wo); add it here
    crow = (np.asarray(bv, np.float32) @ np.asarray(Wo, np.float32)
            + np.asarray(bo, np.float32)).astype(np.float32)
    for core in range(N_CORES):
        b, qi = core // 4, core % 4
        y[b, qi * SQ : (qi + 1) * SQ, :] = results[core]["y"] + crow
    # rows 0..7: all pools masked -> reference uses uniform attention
    for b in range(B):
        vmean = x[b, :LWS, :].astype(np.float64).mean(axis=0).astype(np.float32)
        row = (vmean @ Wv + bv) @ Wo + bo
        y[b, 0:8, :] = row[None, :]
    return y


def kernel(**inputs):
    x = np.asarray(inputs["x"], np.float32)
    lws = int(np.asarray(inputs["local_window_start"]))
    Wq = np.asarray(inputs["Wq"], np.float32)
    bq = np.asarray(inputs["bq"], np.float32)
    Wk = np.asarray(inputs["Wk"], np.float32)
    bk = np.asarray(inputs["bk"], np.float32)
    Wv = np.asarray(inputs["Wv"], np.float32)
    bv = np.asarray(inputs["bv"], np.float32)
    Wo = np.asarray(inputs["Wo"], np.float32)
    bo = np.asarray(inputs["bo"], np.float32)

    if lws != LWS or x.shape != (B, S, D):
        return _numpy_reference(x, lws, Wq, bq, Wk, bk, Wv, bv, Wo, bo)

    try:
        _ensure_path()
        from concourse.bass_utils import run_bass_kernel_spmd

        global _RUNNER
        if _RUNNER is None:
            _RUNNER = build_program()
        nc = _RUNNER

        in_maps = make_in_maps(x, Wq, bq, Wk, bk, Wv, bv, Wo, bo)
        res = run_bass_kernel_spmd(nc, in_maps, list(range(N_CORES)))
        return assemble_output(x, Wv, bv, Wo, bo, res.results)
    except Exception as ex:  # device path unavailable -> correct host fallback
        sys.stderr.write(f"kernel: device path failed ({type(ex).__name__}: {ex}); "
                         "using host fallback\n")
        return _numpy_reference(x, lws, Wq, bq, Wk, bk, Wv, bv, Wo, bo)


if __name__ == "__main__":
    np.random.seed(0)
    xs = np.random.randn(B, S, D).astype(np.float32)
    sc = 1.0 / np.sqrt(D)
    args = dict(
        x=xs,
        local_window_start=LWS,
        Wq=np.random.randn(D, D).astype(np.float32) * sc,
        bq=np.zeros(D, np.float32),
        Wk=np.random.randn(D, D).astype(np.float32) * sc,
        bk=np.zeros(D, np.float32),
        Wv=np.random.randn(D, D).astype(np.float32) * sc,
        bv=np.zeros(D, np.float32),
        Wo=np.random.randn(D, D).astype(np.float32) * sc,
        bo=np.zeros(D, np.float32),
    )
    y = kernel(**args)
    ref = _numpy_reference(
        xs, LWS, args["Wq"], args["bq"], args["Wk"], args["bk"],
        args["Wv"], args["bv"], args["Wo"], args["bo"],
    )
    err = np.abs(y - ref)
    rel = err.max() / np.abs(ref).max()
    print("max abs err:", err.max(), "rel:", rel)
